# revision 3
# baseline (speedup 1.0000x reference)
"""nn_CausalSelfAttention_88854283420050 — Bass/Tile kernel for 8 trn2 cores.

Sharding: tensor-parallel over heads (H=16 -> 2 heads per core).
Each core computes, for its 2 heads: the qkv projection (columns of
c_attn), per-head LayerNorm + RoPE, causal attention, and a partial
output projection y_c = O_heads @ W_proj[:, head cols].T.  The host
sums the 8 partial projections (row-parallel c_proj) and adds b_proj.

Device program (identical SPMD program on all 8 cores, per-core weights):
  Phase A: qkv = x @ Wqkv_c.T (+bias via an appended ones-row of x),
           natural [t, 768] layout; LN stats + apply + RoPE on DVE/ACT;
           PE-transpose of q,k into [c, t] layout; v kept natural [s, c].
  Phase B: per head, per 512-wide t-block: S^T tiles = k_tile^T-stat x q
           (fp32r matmuls), exp on ACT (scores are bounded by sqrt(C)
           after LN so no max-subtraction is needed), causal masking via
           precomputed 0/1 masks on the 4 diagonal tiles, row-sums L via
           a ones-column matmul, O^T accumulation with v stationary,
           normalization by 1/L broadcast through a rank-1 matmul.
  Phase C: y[t, d] partial = sum_h O_h^T-stat x W_proj-cols, DMA out.

All matmuls run in float32r (fp22 mantissa truncation on read, full
fp32 accumulation in PSUM) — full PE rate with ~6e-5 relative error.
"""
import math
import os
import sys

sys.path.insert(0, "/opt/trn_rl_repo")

import numpy as np
from concourse import bacc, mybir, tile
from concourse import bass_utils

T, D, H, C = 2048, 2048, 16, 128
EPS = 1e-6
NCORES = 8
HPC = H // NCORES  # heads per core
DT = 17            # contraction tiles incl. bias row
F32 = mybir.dt.float32
F32R = mybir.dt.float32r
AF = mybir.ActivationFunctionType
ALU = mybir.AluOpType
AX = mybir.AxisListType

NT = T // 128
NB = T // 512

_NC_CACHE = None
LAST_RESULT = None
RUN_KWARGS = {}


def _build_program():
    nc = bacc.Bacc("TRN2", target_bir_lowering=False, debug=False,
                   enable_asserts=True, num_devices=NCORES)

    xT = nc.dram_tensor("xT", [DT * 128, T], F32R, kind="ExternalInput").ap()
    wqkv = nc.dram_tensor("wqkv", [DT * 128, 6 * C], F32R, kind="ExternalInput").ap()
    ropecos = nc.dram_tensor("ropecos", [T, 4 * C], F32, kind="ExternalInput").ap()
    ropesin = nc.dram_tensor("ropesin", [T, 4 * C], F32, kind="ExternalInput").ap()
    masks = nc.dram_tensor("masks", [128, 4 * 512], F32, kind="ExternalInput").ap()
    wp = nc.dram_tensor("wp", [HPC * C, D], F32R, kind="ExternalInput").ap()
    onescol = nc.dram_tensor("onescol", [128, 1], F32R, kind="ExternalInput").ap()
    onesrow = nc.dram_tensor("onesrow", [1, 128], F32R, kind="ExternalInput").ap()
    ident = nc.dram_tensor("ident", [128, 128], F32, kind="ExternalInput").ap()
    y = nc.dram_tensor("y", [T, D], F32, kind="ExternalOutput").ap()

    sc = 1.0 / math.sqrt(C)

    with tile.TileContext(nc) as tc:
        with tc.tile_pool(name="res", bufs=1) as res:
            qT = res.tile([128, HPC, T], F32R, tag="qT")       # [c, h, t]
            kT = res.tile([128, HPC, T], F32R, tag="kT")
            vv = res.tile([128, HPC, NT, C], F32R, tag="vv")   # [s, h, stile, c]
            ot = res.tile([128, HPC, T], F32R, tag="ot")       # [c, h, t]
            ones_c = res.tile([128, 1], F32R, tag="onescol")
            ones_r = res.tile([1, 128], F32R, tag="onesrow")
            id_sb = res.tile([128, 128], F32, tag="ident")

            zeros_c = res.tile([128, 1], F32, tag="zeros_c")
            eps_c = res.tile([128, 1], F32, tag="eps_c")
            nc.gpsimd.memset(zeros_c[:], 0.0)
            nc.gpsimd.memset(eps_c[:], EPS)
            nc.sync.dma_start(ones_c[:], onescol[:])
            nc.sync.dma_start(ones_r[:], onesrow[:])
            nc.sync.dma_start(id_sb[:], ident[:])

            # =========== Phase A: QKV projection + LN + RoPE ===========
            with (
                tc.tile_pool(name="wq", bufs=1) as wqp,
                tc.tile_pool(name="qn", bufs=1) as qnp,
            ):
                qn_all = qnp.tile([128, NT, 4 * C], F32, tag="qn_all")

                w_sb = wqp.tile([128, DT, 6 * C], F32R, tag="w_sb")
                nc.sync.dma_start(
                    w_sb[:], wqkv.rearrange("(a p) n -> p a n", p=128))

                with (
                    tc.tile_pool(name="xcol", bufs=2) as xcolp,
                    tc.tile_pool(name="psA", bufs=2, space="PSUM") as psAp,
                    tc.tile_pool(name="psB", bufs=2, space="PSUM") as psBp,
                    tc.tile_pool(name="rope", bufs=2) as ropep,
                    tc.tile_pool(name="lnst", bufs=2) as lnstp,
                    tc.tile_pool(name="sq", bufs=2) as sqp,
                    tc.tile_pool(name="rot", bufs=2) as rotp,
                ):
                    for tt in range(NT):
                        xcol = xcolp.tile([128, DT, 128], F32R, tag="xcol")
                        nc.sync.dma_start(
                            xcol[:],
                            xT.rearrange("(a p) t -> p a t", p=128)[
                                :, :, tt * 128:(tt + 1) * 128])
                        psA = psAp.tile([128, 512], F32, tag="psA")
                        psB = psBp.tile([128, 256], F32, tag="psB")
                        for dt in range(DT):
                            nc.tensor.matmul(
                                psA[:], xcol[:, dt, :], w_sb[:, dt, 0:512],
                                start=(dt == 0), stop=(dt == DT - 1))
                            nc.tensor.matmul(
                                psB[:], xcol[:, dt, :], w_sb[:, dt, 512:768],
                                start=(dt == 0), stop=(dt == DT - 1))
                        for h in range(HPC):
                            nc.scalar.activation(
                                vv[:, h, tt, :], psB[:, h * C:(h + 1) * C],
                                AF.Copy)
                        rc = ropep.tile([128, 512], F32, tag="ropec")
                        rs = ropep.tile([128, 512], F32, tag="ropes")
                        nc.sync.dma_start(
                            rc[:], ropecos[tt * 128:(tt + 1) * 128, :])
                        nc.sync.dma_start(
                            rs[:], ropesin[tt * 128:(tt + 1) * 128, :])
                        st = lnstp.tile([128, 16], F32, tag="lnst")
                        # st cols: 0:4 sums, 4:8 sumsq, 8:12 rstd, 12:16 -mu*rstd
                        nc.vector.reduce_sum(
                            st[:, 0:4],
                            psA[:].rearrange("p (a b) -> p a b", a=4),
                            axis=AX.X)
                        for i in range(4):
                            sq = sqp.tile([128, 128], F32, tag="sq")
                            nc.scalar.activation(
                                sq[:], psA[:, i * 128:(i + 1) * 128], AF.Square,
                                bias=zeros_c[:], accum_out=st[:, 4 + i:5 + i])
                        mu = lnstp.tile([128, 8], F32, tag="mu")
                        nc.vector.tensor_scalar(
                            mu[:, 0:8], st[:, 0:8], 1.0 / C, None, op0=ALU.mult)
                        var = lnstp.tile([128, 4], F32, tag="var")
                        nc.vector.tensor_tensor(
                            var[:], mu[:, 0:4], mu[:, 0:4], op=ALU.mult)
                        nc.vector.tensor_tensor(
                            var[:], mu[:, 4:8], var[:], op=ALU.subtract)
                        nc.scalar.activation(var[:], var[:], AF.Sqrt,
                                             bias=eps_c[:])
                        nc.vector.reciprocal(st[:, 8:12], var[:])
                        nc.vector.tensor_tensor(
                            st[:, 12:16], mu[:, 0:4], st[:, 8:12], op=ALU.mult)
                        nc.vector.tensor_scalar(
                            st[:, 12:16], st[:, 12:16], -1.0, None,
                            op0=ALU.mult)
                        qn = qn_all[:, tt, :]
                        for i in range(4):
                            nc.vector.tensor_scalar(
                                qn[:, i * 128:(i + 1) * 128],
                                psA[:, i * 128:(i + 1) * 128],
                                st[:, 8 + i:9 + i], st[:, 12 + i:13 + i],
                                op0=ALU.mult, op1=ALU.add)
                        rot = rotp.tile([128, 512], F32, tag="rot")
                        qn3 = qn.rearrange("p (a b) -> p a b", b=2)
                        rot3 = rot[:].rearrange("p (a b) -> p a b", b=2)
                        nc.vector.tensor_scalar(
                            rot3[:, :, 0], qn3[:, :, 1], -1.0, None,
                            op0=ALU.mult)
                        nc.vector.tensor_copy(rot3[:, :, 1], qn3[:, :, 0])
                        nc.vector.tensor_tensor(qn, qn, rc[:], op=ALU.mult)
                        nc.vector.tensor_tensor(
                            rot[:], rot[:], rs[:], op=ALU.mult)
                        nc.vector.tensor_tensor(qn, qn, rot[:], op=ALU.add)

                # ---- Phase A2: transpose q,k into qT/kT ----
                with tc.tile_pool(name="psT", bufs=4, space="PSUM") as psTp:
                    for tt in range(NT):
                        for i in range(4):
                            psT = psTp.tile([128, 128], F32, tag="psT")
                            nc.tensor.transpose(
                                psT[:], qn_all[:, tt, i * 128:(i + 1) * 128],
                                id_sb[:])
                            dst = qT if i < 2 else kT
                            nc.scalar.activation(
                                dst[:, i % 2, tt * 128:(tt + 1) * 128],
                                psT[:], AF.Copy)

            # =========== Phase B: attention per head/t-block ===========
            with tc.tile_pool(name="resB", bufs=1) as resB:
                masks_sb = resB.tile([128, 4 * 512], F32, tag="masks")
                wp_sb = resB.tile([128, HPC, D], F32R, tag="wp")
                nc.sync.dma_start(masks_sb[:], masks[:])
                nc.sync.dma_start(
                    wp_sb[:], wp.rearrange("(h p) d -> p h d", p=128))

                with (
                    tc.tile_pool(name="psS", bufs=3, space="PSUM") as psSp,
                    tc.tile_pool(name="psL", bufs=2, space="PSUM") as psLp,
                    tc.tile_pool(name="psO", bufs=2, space="PSUM") as psOp,
                    tc.tile_pool(name="psBC", bufs=1, space="PSUM") as psBCp,
                    tc.tile_pool(name="aT", bufs=3) as aTp,
                    tc.tile_pool(name="bsm", bufs=2) as bsmp,
                ):
                    for h in range(HPC):
                        for tb in range(NB):
                            S = 4 * (tb + 1)
                            qTs = qT[:, h, tb * 512:(tb + 1) * 512]
                            st_ps = [None] * S

                            def emit_st(s):
                                stp = psSp.tile([128, 512], F32, tag="psS")
                                nc.tensor.matmul(
                                    stp[:], kT[:, h, s * 128:(s + 1) * 128],
                                    qTs, start=True, stop=True)
                                st_ps[s] = stp

                            Lps = psLp.tile([1, 512], F32, tag="psL")
                            Ops = psOp.tile([128, 512], F32, tag="psO")
                            emit_st(0)
                            for s in range(S):
                                if s + 1 < S:
                                    emit_st(s + 1)
                                a = aTp.tile([128, 512], F32R, tag="aT")
                                nc.scalar.activation(
                                    a[:], st_ps[s][:], AF.Exp,
                                    bias=zeros_c[:], scale=sc)
                                st_ps[s] = None
                                if s >= 4 * tb:
                                    j = s - 4 * tb
                                    nc.vector.tensor_tensor(
                                        a[:], a[:],
                                        masks_sb[:, j * 512:(j + 1) * 512],
                                        op=ALU.mult)
                                nc.tensor.matmul(
                                    Lps[:], ones_c[:], a[:],
                                    start=(s == 0), stop=(s == S - 1))
                                nc.tensor.matmul(
                                    Ops[:], vv[:, h, s, :], a[:],
                                    start=(s == 0), stop=(s == S - 1))
                            recL = bsmp.tile([1, 512], F32, tag="recL")
                            nc.vector.reciprocal(recL[:], Lps[:])
                            recLr = bsmp.tile([1, 512], F32R, tag="recLr")
                            nc.scalar.activation(recLr[:], recL[:], AF.Copy)
                            bc = psBCp.tile([128, 512], F32, tag="psBC")
                            nc.tensor.matmul(bc[:], ones_r[:], recLr[:],
                                             start=True, stop=True)
                            bcs = bsmp.tile([128, 512], F32, tag="bcs")
                            nc.scalar.activation(bcs[:], bc[:], AF.Copy)
                            nc.vector.tensor_tensor(
                                ot[:, h, tb * 512:(tb + 1) * 512], Ops[:],
                                bcs[:], op=ALU.mult)

                # =========== Phase C: output projection ===========
                with (
                    tc.tile_pool(name="psY", bufs=2, space="PSUM") as psYp,
                    tc.tile_pool(name="ysb", bufs=3) as ysbp,
                ):
                    for ttt in range(NT):
                        for db in range(NB):
                            yps = psYp.tile([128, 512], F32, tag="psY")
                            for h in range(HPC):
                                nc.tensor.matmul(
                                    yps[:],
                                    ot[:, h, ttt * 128:(ttt + 1) * 128],
                                    wp_sb[:, h, db * 512:(db + 1) * 512],
                                    start=(h == 0), stop=(h == HPC - 1))
                            ysb = ysbp.tile([128, 512], F32, tag="ysb")
                            nc.scalar.activation(ysb[:], yps[:], AF.Copy)
                            nc.sync.dma_start(
                                y[ttt * 128:(ttt + 1) * 128,
                                  db * 512:(db + 1) * 512],
                                ysb[:])

    nc.compile()
    return nc


def _host_prep(x, W_attn, b_attn, W_proj, q_ln_w, k_ln_w):
    f = np.float32
    xT = np.zeros((DT * 128, T), f)
    xT[:D] = x.T
    xT[D] = 1.0

    inv = (1.0 / (10000.0 ** (np.arange(0, C, 2, dtype=f) / C))).astype(f)
    freqs = np.arange(T, dtype=f)[:, None] * inv[None, :]
    sin = np.repeat(np.sin(freqs), 2, axis=1).astype(f)
    cos = np.repeat(np.cos(freqs), 2, axis=1).astype(f)
    part = np.arange(C) ^ 1
    cos_q = cos * q_ln_w[None, :]
    sin_q = sin * q_ln_w[None, part]
    cos_k = cos * k_ln_w[None, :]
    sin_k = sin * k_ln_w[None, part]
    ropecos = np.ascontiguousarray(
        np.concatenate([cos_q, cos_q, cos_k, cos_k], axis=1).astype(f))
    ropesin = np.ascontiguousarray(
        np.concatenate([sin_q, sin_q, sin_k, sin_k], axis=1).astype(f))

    ss = np.arange(128)[:, None]
    ttm = np.arange(512)[None, :]
    masks = np.ascontiguousarray(np.concatenate(
        [(j * 128 + ss <= ttm).astype(f) for j in range(4)], axis=1))

    shared = dict(xT=xT, ropecos=ropecos, ropesin=ropesin, masks=masks,
                  onescol=np.ones((128, 1), f),
                  onesrow=np.ones((1, 128), f),
                  ident=np.eye(128, dtype=f))

    in_maps = []
    for c in range(NCORES):
        h0, h1 = HPC * c, HPC * c + 1
        rows = np.concatenate([
            np.arange(h0 * C, (h0 + 1) * C),
            np.arange(h1 * C, (h1 + 1) * C),
            D + np.arange(h0 * C, (h0 + 1) * C),
            D + np.arange(h1 * C, (h1 + 1) * C),
            2 * D + np.arange(h0 * C, (h0 + 1) * C),
            2 * D + np.arange(h1 * C, (h1 + 1) * C),
        ])
        wqkv = np.zeros((DT * 128, 6 * C), f)
        wqkv[:D] = W_attn[rows].T
        wqkv[D] = b_attn[rows]
        wpc = np.concatenate(
            [W_proj[:, h0 * C:(h0 + 1) * C].T,
             W_proj[:, h1 * C:(h1 + 1) * C].T], axis=0)
        m = dict(shared)
        m["wqkv"] = np.ascontiguousarray(wqkv)
        m["wp"] = np.ascontiguousarray(wpc)
        in_maps.append(m)
    return in_maps


def kernel(x, W_attn, b_attn, W_proj, b_proj, q_ln_w, k_ln_w):
    global _NC_CACHE, LAST_RESULT
    f = np.float32
    x = np.ascontiguousarray(np.asarray(x, f))
    W_attn = np.ascontiguousarray(np.asarray(W_attn, f))
    b_attn = np.ascontiguousarray(np.asarray(b_attn, f))
    W_proj = np.ascontiguousarray(np.asarray(W_proj, f))
    b_proj = np.ascontiguousarray(np.asarray(b_proj, f))
    q_ln_w = np.ascontiguousarray(np.asarray(q_ln_w, f))
    k_ln_w = np.ascontiguousarray(np.asarray(k_ln_w, f))

    in_maps = _host_prep(x, W_attn, b_attn, W_proj, q_ln_w, k_ln_w)
    if _NC_CACHE is None:
        _NC_CACHE = _build_program()
    nc = _NC_CACHE

    res = bass_utils.run_bass_kernel_spmd(
        nc, in_maps, core_ids=list(range(NCORES)),
        trace=bool(os.environ.get("BASS_TRACE")), **RUN_KWARGS)
    LAST_RESULT = res

    y = np.zeros((T, D), np.float32)
    for rmap in res.results:
        y += rmap["y"]
    y += b_proj[None, :]
    return y



# revision 9
# speedup vs baseline: 1.1427x; 1.1427x over previous
"""nn_CausalSelfAttention_88854283420050 — Bass/Tile kernel for 8 trn2 cores.

Sharding: tensor-parallel over heads (H=16 -> 2 heads per core).
Each core computes, for its 2 heads: the qkv projection (columns of
c_attn), per-head LayerNorm + RoPE, causal attention, and a partial
output projection y_c = O_heads @ W_proj[:, head cols].T.  The host
sums the 8 partial projections (row-parallel c_proj) and adds b_proj.

v2: all matmuls in bf16 (1 PE cycle/row vs ~1.5-1.8 for fp32r), LN
stats via bn_stats/bn_aggr, RoPE sign folded into the sin table so the
rotate step is a pure pair-swap, elementwise work spread across
ACT/DVE/GpSimd, q/k transposes inlined into the Phase A t-loop, Phase C
tiles interleaved per 512-t-block right after both heads finish that
block, fp16 output DMA, and host-prearranged DMA layouts (contiguous
per-partition lines).

Device program (identical SPMD program on all 8 cores, per-core weights):
  Phase A (per 128-t tile): qkv = x @ Wqkv_c.T (+bias via an appended
           ones-row of x); bn_stats LN; RoPE; PE-transpose q,k into
           [c, t]; v kept natural [s, c].
  Phase B (per 512-t block, per head): S^T tiles = k_tile^T x q (bf16),
           exp on ACT (bounded scores, no max subtraction), causal 0/1
           masks on diagonal tiles, row-sums L via ones-column matmul,
           O^T accumulation with v stationary, normalization by 1/L
           broadcast through a rank-1 fp32r matmul.
  Phase C (interleaved after each 512-t block): y[t, d] partial =
           sum_h O_h^T x W_proj-cols, fp16 DMA out.
"""
import math
import os
import sys

sys.path.insert(0, "/opt/trn_rl_repo")

import ml_dtypes
import numpy as np
from concourse import bacc, mybir, tile
from concourse import bass_utils

T, D, H, C = 2048, 2048, 16, 128
EPS = 1e-6
NCORES = 8
HPC = H // NCORES  # heads per core
DT = 17            # contraction tiles incl. bias row
F32 = mybir.dt.float32
F32R = mybir.dt.float32r
BF16 = mybir.dt.bfloat16
F16 = mybir.dt.float16
AF = mybir.ActivationFunctionType
ALU = mybir.AluOpType
AX = mybir.AxisListType

NT = T // 128
NB = T // 512

_NC_CACHE = None
LAST_RESULT = None
RUN_KWARGS = {}


def _build_program():
    nc = bacc.Bacc("TRN2", target_bir_lowering=False, debug=False,
                   enable_asserts=True, num_devices=NCORES)

    xprep = nc.dram_tensor("xprep", [128, NT, DT, 128], BF16,
                           kind="ExternalInput").ap()
    wq = nc.dram_tensor("wq", [128, DT, 6 * C], BF16,
                        kind="ExternalInput").ap()
    ropecos = nc.dram_tensor("ropecos", [T, 4 * C], F32,
                             kind="ExternalInput").ap()
    ropesin = nc.dram_tensor("ropesin", [T, 4 * C], F32,
                             kind="ExternalInput").ap()
    masks = nc.dram_tensor("masks", [128, 4 * 512], BF16,
                           kind="ExternalInput").ap()
    wp = nc.dram_tensor("wp", [128, HPC, D], BF16, kind="ExternalInput").ap()
    onescol = nc.dram_tensor("onescol", [128, 1], BF16,
                             kind="ExternalInput").ap()
    onesrow = nc.dram_tensor("onesrow", [1, 128], F32R,
                             kind="ExternalInput").ap()
    ident = nc.dram_tensor("ident", [128, 128], BF16,
                           kind="ExternalInput").ap()
    y = nc.dram_tensor("y", [T, D], F16, kind="ExternalOutput").ap()

    sc = 1.0 / math.sqrt(C)

    with tile.TileContext(nc) as tc:
        with tc.tile_pool(name="res", bufs=1) as res:
            qT = res.tile([128, HPC, T], BF16, tag="qT")       # [c, h, t]
            kT = res.tile([128, HPC, T], BF16, tag="kT")
            vv = res.tile([128, HPC, NT, C], BF16, tag="vv")   # [s, h, st, c]
            ot = res.tile([128, HPC, T], BF16, tag="ot")       # [c, h, t]
            ones_c = res.tile([128, 1], BF16, tag="onescol")
            ones_r = res.tile([1, 128], F32R, tag="onesrow")
            id_sb = res.tile([128, 128], BF16, tag="ident")
            masks_sb = res.tile([128, 4 * 512], BF16, tag="masks")
            wp_sb = res.tile([128, HPC, D], BF16, tag="wp")
            w_sb = res.tile([128, DT, 6 * C], BF16, tag="w_sb")

            zeros_c = res.tile([128, 1], F32, tag="zeros_c")
            eps_c = res.tile([128, 1], F32, tag="eps_c")
            nc.gpsimd.memset(zeros_c[:], 0.0)
            nc.gpsimd.memset(eps_c[:], EPS)
            nc.sync.dma_start(ones_c[:], onescol[:])
            nc.sync.dma_start(ones_r[:], onesrow[:])
            nc.sync.dma_start(id_sb[:], ident[:])
            nc.sync.dma_start(masks_sb[:], masks[:])
            nc.sync.dma_start(wp_sb[:], wp[:])
            # split the weight load so the first matmuls start early
            for i in range(4):
                d0 = (DT * i) // 4
                d1 = (DT * (i + 1)) // 4
                nc.sync.dma_start(w_sb[:, d0:d1, :], wq[:, d0:d1, :])

            # =========== Phase A: QKV projection + LN + RoPE ===========
            with (
                tc.tile_pool(name="xcol", bufs=2) as xcolp,
                tc.tile_pool(name="psA", bufs=2, space="PSUM") as psAp,
                tc.tile_pool(name="psB", bufs=2, space="PSUM") as psBp,
                tc.tile_pool(name="psT", bufs=4, space="PSUM") as psTp,
                tc.tile_pool(name="rope", bufs=2) as ropep,
                tc.tile_pool(name="lnst", bufs=2) as lnstp,
                tc.tile_pool(name="qn", bufs=2) as qnp,
                tc.tile_pool(name="rot", bufs=2) as rotp,
                tc.tile_pool(name="qn16", bufs=2) as qn16p,
            ):
                for tt in range(NT):
                    xcol = xcolp.tile([128, DT, 128], BF16, tag="xcol")
                    nc.sync.dma_start(xcol[:], xprep[:, tt])
                    rc = ropep.tile([128, 512], F32, tag="ropec")
                    rs = ropep.tile([128, 512], F32, tag="ropes")
                    nc.sync.dma_start(rc[:], ropecos[tt * 128:(tt + 1) * 128, :])
                    nc.sync.dma_start(rs[:], ropesin[tt * 128:(tt + 1) * 128, :])

                    psA = psAp.tile([128, 512], F32, tag="psA")
                    psB = psBp.tile([128, 256], F32, tag="psB")
                    for dt in range(DT):
                        nc.tensor.matmul(
                            psA[:], xcol[:, dt, :], w_sb[:, dt, 0:512],
                            start=(dt == 0), stop=(dt == DT - 1))
                        nc.tensor.matmul(
                            psB[:], xcol[:, dt, :], w_sb[:, dt, 512:768],
                            start=(dt == 0), stop=(dt == DT - 1))
                    # v -> SBUF bf16 (ACT)
                    for h in range(HPC):
                        nc.scalar.activation(
                            vv[:, h, tt, :], psB[:, h * C:(h + 1) * C],
                            AF.Copy)
                    # LN stats via bn_stats/bn_aggr (DVE)
                    bst = lnstp.tile([128, 4, 6], F32, tag="bst")
                    agg = lnstp.tile([128, 4, 2], F32, tag="agg")
                    for i in range(4):
                        nc.vector.bn_stats(
                            bst[:, i, :], psA[:, i * 128:(i + 1) * 128])
                        nc.vector.bn_aggr(agg[:, i, :], bst[:, i, :])
                    std = lnstp.tile([128, 4], F32, tag="std")
                    rstd = lnstp.tile([128, 4], F32, tag="rstd")
                    nm = lnstp.tile([128, 4], F32, tag="nm")
                    nc.scalar.activation(std[:], agg[:, :, 1], AF.Sqrt,
                                         bias=eps_c[:])
                    nc.vector.reciprocal(rstd[:], std[:])
                    # nm = -mean * rstd
                    nc.vector.scalar_tensor_tensor(
                        nm[:], agg[:, :, 0], -1.0, rstd[:],
                        op0=ALU.mult, op1=ALU.mult)
                    # qn = psA*rstd + nm (f32)
                    qn = qnp.tile([128, 512], F32, tag="qn")
                    for i in range(4):
                        nc.vector.tensor_scalar(
                            qn[:, i * 128:(i + 1) * 128],
                            psA[:, i * 128:(i + 1) * 128],
                            rstd[:, i:i + 1], nm[:, i:i + 1],
                            op0=ALU.mult, op1=ALU.add)
                    # rope: rot = pair-swap(qn) (GpSimd), sign is in rs
                    rot = rotp.tile([128, 512], F32, tag="rot")
                    qn3 = qn[:].rearrange("p (a b) -> p a b", b=2)
                    rot3 = rot[:].rearrange("p (a b) -> p a b", b=2)
                    nc.gpsimd.tensor_copy(rot3[:, :, 0], qn3[:, :, 1])
                    nc.gpsimd.tensor_copy(rot3[:, :, 1], qn3[:, :, 0])
                    qc = qnp.tile([128, 512], F32, tag="qc")
                    nc.vector.tensor_tensor(qc[:], qn[:], rc[:], op=ALU.mult)
                    nc.gpsimd.tensor_tensor(rot[:], rot[:], rs[:],
                                            op=ALU.mult)
                    qn16 = qn16p.tile([128, 512], BF16, tag="qn16")
                    nc.vector.tensor_tensor(qn16[:], qc[:], rot[:],
                                            op=ALU.add)
                    # transpose q,k into [c, t] (PE) + ACT copies out
                    for i in range(4):
                        psT = psTp.tile([128, 128], BF16, tag="psT")
                        nc.tensor.transpose(
                            psT[:], qn16[:, i * 128:(i + 1) * 128], id_sb[:])
                        dst = qT if i < 2 else kT
                        nc.scalar.activation(
                            dst[:, i % 2, tt * 128:(tt + 1) * 128],
                            psT[:], AF.Copy)

            # ====== Phase B + C: attention, interleaved projection ======
            with (
                tc.tile_pool(name="psS", bufs=3, space="PSUM") as psSp,
                tc.tile_pool(name="psL", bufs=1, space="PSUM") as psLp,
                tc.tile_pool(name="psO", bufs=2, space="PSUM") as psOp,
                tc.tile_pool(name="psY", bufs=2, space="PSUM") as psYp,
                tc.tile_pool(name="aT", bufs=3) as aTp,
                tc.tile_pool(name="bsm", bufs=2) as bsmp,
                tc.tile_pool(name="ysb", bufs=3) as ysbp,
            ):
                for tb in range(NB):
                    S = 4 * (tb + 1)
                    for h in range(HPC):
                        qTs = qT[:, h, tb * 512:(tb + 1) * 512]
                        st_ps = [None] * S

                        def emit_st(s):
                            stp = psSp.tile([128, 512], F32, tag="psS")
                            nc.tensor.matmul(
                                stp[:], kT[:, h, s * 128:(s + 1) * 128],
                                qTs, start=True, stop=True)
                            st_ps[s] = stp

                        Lps = psLp.tile([1, 512], F32, tag="psL")
                        Ops = psOp.tile([128, 512], F32, tag="psO")
                        emit_st(0)
                        if S > 1:
                            emit_st(1)
                        for s in range(S):
                            if s + 2 < S:
                                emit_st(s + 2)
                            a = aTp.tile([128, 512], BF16, tag="aT")
                            nc.scalar.activation(a[:], st_ps[s][:], AF.Exp,
                                                 bias=zeros_c[:], scale=sc)
                            st_ps[s] = None
                            if s >= 4 * tb:
                                j = s - 4 * tb
                                nc.vector.tensor_tensor(
                                    a[:], a[:],
                                    masks_sb[:, j * 512:(j + 1) * 512],
                                    op=ALU.mult)
                            nc.tensor.matmul(
                                Lps[:], ones_c[:], a[:],
                                start=(s == 0), stop=(s == S - 1))
                            nc.tensor.matmul(
                                Ops[:], vv[:, h, s, :], a[:],
                                start=(s == 0), stop=(s == S - 1))
                        recL = bsmp.tile([1, 512], F32, tag="recL")
                        nc.vector.reciprocal(recL[:], Lps[:])
                        recLr = bsmp.tile([1, 512], F32R, tag="recLr")
                        nc.scalar.activation(recLr[:], recL[:], AF.Copy)
                        bc = psYp.tile([128, 512], F32, tag="psY")
                        nc.tensor.matmul(bc[:], ones_r[:], recLr[:],
                                         start=True, stop=True)
                        bcs = bsmp.tile([128, 512], F32, tag="bcs")
                        nc.scalar.activation(bcs[:], bc[:], AF.Copy)
                        nc.vector.tensor_tensor(
                            ot[:, h, tb * 512:(tb + 1) * 512], Ops[:],
                            bcs[:], op=ALU.mult)

                    # ---- Phase C for this t-block ----
                    for ti in range(4):
                        ttt = tb * 4 + ti
                        for db in range(NB):
                            yps = psYp.tile([128, 512], F32, tag="psY")
                            for h in range(HPC):
                                nc.tensor.matmul(
                                    yps[:],
                                    ot[:, h, ttt * 128:(ttt + 1) * 128],
                                    wp_sb[:, h, db * 512:(db + 1) * 512],
                                    start=(h == 0), stop=(h == HPC - 1))
                            ysb = ysbp.tile([128, 512], F16, tag="ysb")
                            if (ttt * NB + db) % 2 == 0:
                                nc.scalar.activation(ysb[:], yps[:], AF.Copy)
                            else:
                                nc.vector.tensor_copy(ysb[:], yps[:])
                            nc.sync.dma_start(
                                y[ttt * 128:(ttt + 1) * 128,
                                  db * 512:(db + 1) * 512],
                                ysb[:])

    nc.compile()
    return nc


def _host_prep(x, W_attn, b_attn, W_proj, q_ln_w, k_ln_w):
    f = np.float32
    bf = ml_dtypes.bfloat16
    xT = np.zeros((DT * 128, T), f)
    xT[:D] = x.T
    xT[D] = 1.0
    xprep = np.ascontiguousarray(
        xT.reshape(DT, 128, NT, 128).transpose(1, 2, 0, 3).astype(bf))

    inv = (1.0 / (10000.0 ** (np.arange(0, C, 2, dtype=f) / C))).astype(f)
    freqs = np.arange(T, dtype=f)[:, None] * inv[None, :]
    sin = np.repeat(np.sin(freqs), 2, axis=1).astype(f)
    cos = np.repeat(np.cos(freqs), 2, axis=1).astype(f)
    part = np.arange(C) ^ 1
    sign = np.where(np.arange(C) % 2 == 0, -1.0, 1.0).astype(f)
    cos_q = cos * q_ln_w[None, :]
    sin_q = sin * q_ln_w[None, part] * sign[None, :]
    cos_k = cos * k_ln_w[None, :]
    sin_k = sin * k_ln_w[None, part] * sign[None, :]
    ropecos = np.ascontiguousarray(
        np.concatenate([cos_q, cos_q, cos_k, cos_k], axis=1).astype(f))
    ropesin = np.ascontiguousarray(
        np.concatenate([sin_q, sin_q, sin_k, sin_k], axis=1).astype(f))

    ss = np.arange(128)[:, None]
    ttm = np.arange(512)[None, :]
    masks = np.ascontiguousarray(np.concatenate(
        [(j * 128 + ss <= ttm).astype(f) for j in range(4)],
        axis=1).astype(bf))

    shared = dict(xprep=xprep, ropecos=ropecos, ropesin=ropesin, masks=masks,
                  onescol=np.ones((128, 1), bf),
                  onesrow=np.ones((1, 128), f),
                  ident=np.eye(128, dtype=bf))

    in_maps = []
    for c in range(NCORES):
        h0, h1 = HPC * c, HPC * c + 1
        rows = np.concatenate([
            np.arange(h0 * C, (h0 + 1) * C),
            np.arange(h1 * C, (h1 + 1) * C),
            D + np.arange(h0 * C, (h0 + 1) * C),
            D + np.arange(h1 * C, (h1 + 1) * C),
            2 * D + np.arange(h0 * C, (h0 + 1) * C),
            2 * D + np.arange(h1 * C, (h1 + 1) * C),
        ])
        wqkv = np.zeros((DT * 128, 6 * C), f)
        wqkv[:D] = W_attn[rows].T
        wqkv[D] = b_attn[rows]
        wqc = np.ascontiguousarray(
            wqkv.reshape(DT, 128, 6 * C).transpose(1, 0, 2).astype(bf))
        wpc = np.concatenate(
            [W_proj[:, h0 * C:(h0 + 1) * C].T,
             W_proj[:, h1 * C:(h1 + 1) * C].T], axis=0)
        wpc = np.ascontiguousarray(
            wpc.reshape(HPC, 128, D).transpose(1, 0, 2).astype(bf))
        m = dict(shared)
        m["wq"] = wqc
        m["wp"] = wpc
        in_maps.append(m)
    return in_maps


def kernel(x, W_attn, b_attn, W_proj, b_proj, q_ln_w, k_ln_w):
    global _NC_CACHE, LAST_RESULT
    f = np.float32
    x = np.ascontiguousarray(np.asarray(x, f))
    W_attn = np.ascontiguousarray(np.asarray(W_attn, f))
    b_attn = np.ascontiguousarray(np.asarray(b_attn, f))
    W_proj = np.ascontiguousarray(np.asarray(W_proj, f))
    b_proj = np.ascontiguousarray(np.asarray(b_proj, f))
    q_ln_w = np.ascontiguousarray(np.asarray(q_ln_w, f))
    k_ln_w = np.ascontiguousarray(np.asarray(k_ln_w, f))

    in_maps = _host_prep(x, W_attn, b_attn, W_proj, q_ln_w, k_ln_w)
    if _NC_CACHE is None:
        _NC_CACHE = _build_program()
    nc = _NC_CACHE

    res = bass_utils.run_bass_kernel_spmd(
        nc, in_maps, core_ids=list(range(NCORES)),
        trace=bool(os.environ.get("BASS_TRACE")), **RUN_KWARGS)
    LAST_RESULT = res

    y = np.zeros((T, D), np.float32)
    for rmap in res.results:
        y += rmap["y"].astype(np.float32)
    y += b_proj[None, :]
    return y


# revision 10
# speedup vs baseline: 1.2080x; 1.0572x over previous
"""nn_CausalSelfAttention_88854283420050 — Bass/Tile kernel for 8 trn2 cores.

Sharding: tensor-parallel over heads (H=16 -> 2 heads per core).
Each core computes, for its 2 heads: the qkv projection (columns of
c_attn), per-head LayerNorm + RoPE, causal attention, and a partial
output projection y_c = O_heads @ W_proj[:, head cols].T.  The host
sums the 8 partial projections (row-parallel c_proj) and adds b_proj
(plus the exact v-bias term b_v @ W_proj.T, so the v matmul can skip
the bias row).

v3: bf16 matmuls (PE ~1 cycle/row), DMA issue order prioritizing the
first tiles' data (weights per-dt, masks/wp deferred), q/k transposes
software-pipelined one t-tile behind the GEMMs so the LN+RoPE chain
latency hides under the next tile's matmuls, softmax normalization via
broadcast-then-fat-reciprocal (no single-partition DVE ops in the
critical path), row-batched fp16 y DMAs (16 issues instead of 64), and
Phase C interleaved per 512-t block.
"""
import math
import os
import sys

sys.path.insert(0, "/opt/trn_rl_repo")

import ml_dtypes
import numpy as np
from concourse import bacc, mybir, tile
from concourse import bass_utils

T, D, H, C = 2048, 2048, 16, 128
EPS = 1e-6
NCORES = 8
HPC = H // NCORES  # heads per core
DT = 17            # q/k contraction tiles incl. bias row
DTV = 16           # v contraction tiles (bias folded on host)
F32 = mybir.dt.float32
F32R = mybir.dt.float32r
BF16 = mybir.dt.bfloat16
F16 = mybir.dt.float16
AF = mybir.ActivationFunctionType
ALU = mybir.AluOpType
AX = mybir.AxisListType

NT = T // 128
NB = T // 512

_NC_CACHE = None
LAST_RESULT = None
RUN_KWARGS = {}


def _build_program():
    nc = bacc.Bacc("TRN2", target_bir_lowering=False, debug=False,
                   enable_asserts=True, num_devices=NCORES)

    xprep = nc.dram_tensor("xprep", [128, NT, DT, 128], BF16,
                           kind="ExternalInput").ap()
    wq = nc.dram_tensor("wq", [128, DT, 6 * C], BF16,
                        kind="ExternalInput").ap()
    ropecos = nc.dram_tensor("ropecos", [T, 4 * C], F32,
                             kind="ExternalInput").ap()
    ropesin = nc.dram_tensor("ropesin", [T, 4 * C], F32,
                             kind="ExternalInput").ap()
    masks = nc.dram_tensor("masks", [128, 4 * 512], BF16,
                           kind="ExternalInput").ap()
    wp = nc.dram_tensor("wp", [128, HPC, D], BF16, kind="ExternalInput").ap()
    onescol = nc.dram_tensor("onescol", [128, 1], BF16,
                             kind="ExternalInput").ap()
    onesrow = nc.dram_tensor("onesrow", [1, 128], F32R,
                             kind="ExternalInput").ap()
    ident = nc.dram_tensor("ident", [128, 128], BF16,
                           kind="ExternalInput").ap()
    y = nc.dram_tensor("y", [T, D], F16, kind="ExternalOutput").ap()

    sc = 1.0 / math.sqrt(C)

    with tile.TileContext(nc) as tc:
        with tc.tile_pool(name="res", bufs=1) as res:
            qT = res.tile([128, HPC, T], BF16, tag="qT")       # [c, h, t]
            kT = res.tile([128, HPC, T], BF16, tag="kT")
            vv = res.tile([128, HPC, NT, C], BF16, tag="vv")   # [s, h, st, c]
            ot = res.tile([128, HPC, T], BF16, tag="ot")       # [c, h, t]
            ones_c = res.tile([128, 1], BF16, tag="onescol")
            ones_r = res.tile([1, 128], F32R, tag="onesrow")
            id_sb = res.tile([128, 128], BF16, tag="ident")
            masks_sb = res.tile([128, 4 * 512], BF16, tag="masks")
            wp_sb = res.tile([128, HPC, D], BF16, tag="wp")
            w_sb = res.tile([128, DT, 6 * C], BF16, tag="w_sb")

            zeros_c = res.tile([128, 1], F32, tag="zeros_c")
            eps_c = res.tile([128, 1], F32, tag="eps_c")
            nc.gpsimd.memset(zeros_c[:], 0.0)
            nc.gpsimd.memset(eps_c[:], EPS)
            nc.sync.dma_start(ones_c[:], onescol[:])
            nc.sync.dma_start(ones_r[:], onesrow[:])
            nc.sync.dma_start(id_sb[:], ident[:])
            # first three dt-chunks of the weights ahead of everything else
            for dt in range(3):
                nc.sync.dma_start(w_sb[:, dt, :], wq[:, dt, :])

            # =========== Phase A: QKV projection + LN + RoPE ===========
            with (
                tc.tile_pool(name="xcol", bufs=2) as xcolp,
                tc.tile_pool(name="psA", bufs=2, space="PSUM") as psAp,
                tc.tile_pool(name="psB", bufs=2, space="PSUM") as psBp,
                tc.tile_pool(name="psT", bufs=4, space="PSUM") as psTp,
                tc.tile_pool(name="rope", bufs=2) as ropep,
                tc.tile_pool(name="lnst", bufs=2) as lnstp,
                tc.tile_pool(name="qn", bufs=2) as qnp,
                tc.tile_pool(name="rot", bufs=2) as rotp,
                tc.tile_pool(name="qn16", bufs=2) as qn16p,
            ):
                def emit_transposes(qn16, tt):
                    for i in range(4):
                        psT = psTp.tile([128, 128], BF16, tag="psT")
                        nc.tensor.transpose(
                            psT[:], qn16[:, i * 128:(i + 1) * 128], id_sb[:])
                        dst = qT if i < 2 else kT
                        nc.scalar.activation(
                            dst[:, i % 2, tt * 128:(tt + 1) * 128],
                            psT[:], AF.Copy)

                prev = None
                for tt in range(NT):
                    xcol = xcolp.tile([128, DT, 128], BF16, tag="xcol")
                    nc.sync.dma_start(xcol[:], xprep[:, tt])
                    rc = ropep.tile([128, 512], F32, tag="ropec")
                    rs = ropep.tile([128, 512], F32, tag="ropes")
                    nc.sync.dma_start(rc[:], ropecos[tt * 128:(tt + 1) * 128, :])
                    nc.sync.dma_start(rs[:], ropesin[tt * 128:(tt + 1) * 128, :])
                    if tt == 0:
                        for dt in range(3, DT):
                            nc.sync.dma_start(w_sb[:, dt, :], wq[:, dt, :])
                    if tt == 4:
                        nc.sync.dma_start(masks_sb[:], masks[:])
                        nc.sync.dma_start(wp_sb[:], wp[:])

                    psA = psAp.tile([128, 512], F32, tag="psA")
                    psB = psBp.tile([128, 256], F32, tag="psB")
                    for dt in range(DT):
                        nc.tensor.matmul(
                            psA[:], xcol[:, dt, :], w_sb[:, dt, 0:512],
                            start=(dt == 0), stop=(dt == DT - 1))
                        if dt < DTV:
                            nc.tensor.matmul(
                                psB[:], xcol[:, dt, :], w_sb[:, dt, 512:768],
                                start=(dt == 0), stop=(dt == DTV - 1))
                    # v -> SBUF bf16 (ACT)
                    for h in range(HPC):
                        nc.scalar.activation(
                            vv[:, h, tt, :], psB[:, h * C:(h + 1) * C],
                            AF.Copy)
                    # LN stats via bn_stats/bn_aggr (DVE)
                    bst = lnstp.tile([128, 4, 6], F32, tag="bst")
                    agg = lnstp.tile([128, 4, 2], F32, tag="agg")
                    for i in range(4):
                        nc.vector.bn_stats(
                            bst[:, i, :], psA[:, i * 128:(i + 1) * 128])
                        nc.vector.bn_aggr(agg[:, i, :], bst[:, i, :])
                    std = lnstp.tile([128, 4], F32, tag="std")
                    rstd = lnstp.tile([128, 4], F32, tag="rstd")
                    nm = lnstp.tile([128, 4], F32, tag="nm")
                    nc.scalar.activation(std[:], agg[:, :, 1], AF.Sqrt,
                                         bias=eps_c[:])
                    nc.vector.reciprocal(rstd[:], std[:])
                    # nm = -mean * rstd
                    nc.vector.scalar_tensor_tensor(
                        nm[:], agg[:, :, 0], -1.0, rstd[:],
                        op0=ALU.mult, op1=ALU.mult)
                    # qn = psA*rstd + nm (f32)
                    qn = qnp.tile([128, 512], F32, tag="qn")
                    for i in range(4):
                        nc.vector.tensor_scalar(
                            qn[:, i * 128:(i + 1) * 128],
                            psA[:, i * 128:(i + 1) * 128],
                            rstd[:, i:i + 1], nm[:, i:i + 1],
                            op0=ALU.mult, op1=ALU.add)
                    # rope: rot = pair-swap(qn), sign is folded into rs
                    rot = rotp.tile([128, 512], F32, tag="rot")
                    qn3 = qn[:].rearrange("p (a b) -> p a b", b=2)
                    rot3 = rot[:].rearrange("p (a b) -> p a b", b=2)
                    last = (tt == NT - 1)
                    if last:
                        # keep the tail chain off the slow gpsimd ops
                        nc.scalar.activation(rot3[:, :, 0], qn3[:, :, 1],
                                             AF.Copy)
                        nc.scalar.activation(rot3[:, :, 1], qn3[:, :, 0],
                                             AF.Copy)
                    else:
                        nc.gpsimd.tensor_copy(rot3[:, :, 0], qn3[:, :, 1])
                        nc.gpsimd.tensor_copy(rot3[:, :, 1], qn3[:, :, 0])
                    qc = qnp.tile([128, 512], F32, tag="qc")
                    nc.vector.tensor_tensor(qc[:], qn[:], rc[:], op=ALU.mult)
                    if last:
                        nc.vector.tensor_tensor(rot[:], rot[:], rs[:],
                                                op=ALU.mult)
                    else:
                        nc.gpsimd.tensor_tensor(rot[:], rot[:], rs[:],
                                                op=ALU.mult)
                    qn16 = qn16p.tile([128, 512], BF16, tag="qn16")
                    nc.vector.tensor_tensor(qn16[:], qc[:], rot[:],
                                            op=ALU.add)
                    if prev is not None:
                        emit_transposes(*prev)
                    prev = (qn16, tt)
                emit_transposes(*prev)

            # ====== Phase B + C: attention, interleaved projection ======
            with (
                tc.tile_pool(name="psS", bufs=2, space="PSUM") as psSp,
                tc.tile_pool(name="psL", bufs=2, space="PSUM") as psLp,
                tc.tile_pool(name="psO", bufs=2, space="PSUM") as psOp,
                tc.tile_pool(name="psY", bufs=2, space="PSUM") as psYp,
                tc.tile_pool(name="aT", bufs=3) as aTp,
                tc.tile_pool(name="bsm", bufs=2) as bsmp,
                tc.tile_pool(name="ysb", bufs=2) as ysbp,
            ):
                for tb in range(NB):
                    S = 4 * (tb + 1)
                    for h in range(HPC):
                        qTs = qT[:, h, tb * 512:(tb + 1) * 512]
                        st_ps = [None] * S

                        def emit_st(s):
                            stp = psSp.tile([128, 512], F32, tag="psS")
                            nc.tensor.matmul(
                                stp[:], kT[:, h, s * 128:(s + 1) * 128],
                                qTs, start=True, stop=True)
                            st_ps[s] = stp

                        Lps = psLp.tile([1, 512], F32, tag="psL")
                        Ops = psOp.tile([128, 512], F32, tag="psO")
                        emit_st(0)
                        for s in range(S):
                            if s + 1 < S:
                                emit_st(s + 1)
                            a = aTp.tile([128, 512], BF16, tag="aT")
                            nc.scalar.activation(a[:], st_ps[s][:], AF.Exp,
                                                 bias=zeros_c[:], scale=sc)
                            st_ps[s] = None
                            if s >= 4 * tb:
                                j = s - 4 * tb
                                nc.vector.tensor_tensor(
                                    a[:], a[:],
                                    masks_sb[:, j * 512:(j + 1) * 512],
                                    op=ALU.mult)
                            nc.tensor.matmul(
                                Lps[:], ones_c[:], a[:],
                                start=(s == 0), stop=(s == S - 1))
                            nc.tensor.matmul(
                                Ops[:], vv[:, h, s, :], a[:],
                                start=(s == 0), stop=(s == S - 1))
                        # normalization: broadcast L, then fat reciprocal
                        Lrow = bsmp.tile([1, 512], F32R, tag="Lrow")
                        nc.scalar.activation(Lrow[:], Lps[:], AF.Copy)
                        bc = psYp.tile([128, 512], F32, tag="psY")
                        nc.tensor.matmul(bc[:], ones_r[:], Lrow[:],
                                         start=True, stop=True)
                        bcs = bsmp.tile([128, 512], F32, tag="bcs")
                        nc.vector.reciprocal(bcs[:], bc[:])
                        nc.vector.tensor_tensor(
                            ot[:, h, tb * 512:(tb + 1) * 512], Ops[:],
                            bcs[:], op=ALU.mult)

                    # ---- Phase C for this t-block ----
                    for ti in range(4):
                        ttt = tb * 4 + ti
                        yrow = ysbp.tile([128, NB, 512], F16, tag="ysb")
                        for db in range(NB):
                            yps = psYp.tile([128, 512], F32, tag="psY")
                            for h in range(HPC):
                                nc.tensor.matmul(
                                    yps[:],
                                    ot[:, h, ttt * 128:(ttt + 1) * 128],
                                    wp_sb[:, h, db * 512:(db + 1) * 512],
                                    start=(h == 0), stop=(h == HPC - 1))
                            if db % 2 == 0:
                                nc.scalar.activation(yrow[:, db, :], yps[:],
                                                     AF.Copy)
                            else:
                                nc.vector.tensor_copy(yrow[:, db, :], yps[:])
                        nc.sync.dma_start(
                            y[ttt * 128:(ttt + 1) * 128, :],
                            yrow[:].rearrange("p a b -> p (a b)"))

    nc.compile()
    return nc


def _host_prep(x, W_attn, b_attn, W_proj, q_ln_w, k_ln_w):
    f = np.float32
    bf = ml_dtypes.bfloat16
    xT = np.zeros((DT * 128, T), f)
    xT[:D] = x.T
    xT[D] = 1.0
    xprep = np.ascontiguousarray(
        xT.reshape(DT, 128, NT, 128).transpose(1, 2, 0, 3).astype(bf))

    inv = (1.0 / (10000.0 ** (np.arange(0, C, 2, dtype=f) / C))).astype(f)
    freqs = np.arange(T, dtype=f)[:, None] * inv[None, :]
    sin = np.repeat(np.sin(freqs), 2, axis=1).astype(f)
    cos = np.repeat(np.cos(freqs), 2, axis=1).astype(f)
    part = np.arange(C) ^ 1
    sign = np.where(np.arange(C) % 2 == 0, -1.0, 1.0).astype(f)
    cos_q = cos * q_ln_w[None, :]
    sin_q = sin * q_ln_w[None, part] * sign[None, :]
    cos_k = cos * k_ln_w[None, :]
    sin_k = sin * k_ln_w[None, part] * sign[None, :]
    ropecos = np.ascontiguousarray(
        np.concatenate([cos_q, cos_q, cos_k, cos_k], axis=1).astype(f))
    ropesin = np.ascontiguousarray(
        np.concatenate([sin_q, sin_q, sin_k, sin_k], axis=1).astype(f))

    ss = np.arange(128)[:, None]
    ttm = np.arange(512)[None, :]
    masks = np.ascontiguousarray(np.concatenate(
        [(j * 128 + ss <= ttm).astype(f) for j in range(4)],
        axis=1).astype(bf))

    shared = dict(xprep=xprep, ropecos=ropecos, ropesin=ropesin, masks=masks,
                  onescol=np.ones((128, 1), bf),
                  onesrow=np.ones((1, 128), f),
                  ident=np.eye(128, dtype=bf))

    in_maps = []
    for c in range(NCORES):
        h0, h1 = HPC * c, HPC * c + 1
        rows = np.concatenate([
            np.arange(h0 * C, (h0 + 1) * C),
            np.arange(h1 * C, (h1 + 1) * C),
            D + np.arange(h0 * C, (h0 + 1) * C),
            D + np.arange(h1 * C, (h1 + 1) * C),
            2 * D + np.arange(h0 * C, (h0 + 1) * C),
            2 * D + np.arange(h1 * C, (h1 + 1) * C),
        ])
        wqkv = np.zeros((DT * 128, 6 * C), f)
        wqkv[:D] = W_attn[rows].T
        wqkv[D] = b_attn[rows]
        wqkv[D, 512:768] = 0.0  # v bias folded on host
        wqc = np.ascontiguousarray(
            wqkv.reshape(DT, 128, 6 * C).transpose(1, 0, 2).astype(bf))
        wpc = np.concatenate(
            [W_proj[:, h0 * C:(h0 + 1) * C].T,
             W_proj[:, h1 * C:(h1 + 1) * C].T], axis=0)
        wpc = np.ascontiguousarray(
            wpc.reshape(HPC, 128, D).transpose(1, 0, 2).astype(bf))
        m = dict(shared)
        m["wq"] = wqc
        m["wp"] = wpc
        in_maps.append(m)
    return in_maps


def kernel(x, W_attn, b_attn, W_proj, b_proj, q_ln_w, k_ln_w):
    global _NC_CACHE, LAST_RESULT
    f = np.float32
    x = np.ascontiguousarray(np.asarray(x, f))
    W_attn = np.ascontiguousarray(np.asarray(W_attn, f))
    b_attn = np.ascontiguousarray(np.asarray(b_attn, f))
    W_proj = np.ascontiguousarray(np.asarray(W_proj, f))
    b_proj = np.ascontiguousarray(np.asarray(b_proj, f))
    q_ln_w = np.ascontiguousarray(np.asarray(q_ln_w, f))
    k_ln_w = np.ascontiguousarray(np.asarray(k_ln_w, f))

    in_maps = _host_prep(x, W_attn, b_attn, W_proj, q_ln_w, k_ln_w)
    if _NC_CACHE is None:
        _NC_CACHE = _build_program()
    nc = _NC_CACHE

    res = bass_utils.run_bass_kernel_spmd(
        nc, in_maps, core_ids=list(range(NCORES)),
        trace=bool(os.environ.get("BASS_TRACE")), **RUN_KWARGS)
    LAST_RESULT = res

    y = np.zeros((T, D), np.float32)
    for rmap in res.results:
        y += rmap["y"].astype(np.float32)
    # v bias contribution (exact): O picks up +b_v since attn rows sum to 1
    y += b_attn[2 * D:] @ W_proj.T + b_proj
    return y
